# revision 4
# baseline (speedup 1.0000x reference)
"""CP-decomposed conv (pointwise -> depthwise-h -> depthwise-w -> pointwise)
as a Bass/Tile kernel on 8 TRN2 NeuronCores.

Strategy:
  - Data-parallel over batch: 32 images -> 4 per core, no collectives.
  - fp16 wire format: x and out cross HBM as fp16; fp32 accumulation in PSUM.
  - Stage A+B: pointwise C->R with the depthwise h-conv folded in:
      y2[r,i,w] = sum_{h,c} (factor3[c,r]*factor1[h,r]) * x[c,i+h,w]
    6 accumulating fp16 matmuls per PSUM bank (3 h-shifts x 2 C-chunks).
    psA tiles span 2 PSUM banks (10 output rows) so the stage-C elementwise
    ops run at ~940 elements each instead of ~470 (fixed-cost amortization).
  - Stage C: depthwise w-conv straight out of PSUM with per-partition
    scalars (factor2[w,r] on partition r): 1 ACT copy-scale + 2 DVE STT.
  - Stage D: projection R->F; psD tiles span 2 banks so each PSUM->SBUF
    copy moves 1024 elements; copies split ~30/70 between DVE and ACT to
    balance the two engines (DVE also owns the 2 STT passes of stage C).
  - DMA: whole half-image input loads (2.4 MB, SWDGE on GpSimd), one
    output DMA per (half-image, f-chunk) (1.13 MB, HWDGE on SP).
  - Stage-D matmuls of half-image h are emitted after stage-A of h+1 so
    the PE never waits on the stage-C tail (keeps HAM un-throttled).
"""

import sys
import numpy as np

for _p in ("/opt/trn_rl_repo",):
    if _p not in sys.path:
        sys.path.insert(0, _p)

B, C, H, W = 32, 256, 96, 96
F, FH, FW, R = 512, 3, 3, 128
OH, OW = H - FH + 1, W - FW + 1  # 94, 94
NCORES = 8
BLOC = B // NCORES  # 4 images per core

SH = 47            # output rows per half-image
XROWS = SH + 2     # x rows needed per half-image
ROW_TILES = [(0, 10), (10, 10), (20, 10), (30, 10), (40, 7)]
COL_CHUNKS = [(0, 1024), (1024, 1024), (2048, 1024), (3072, 1024), (4096, 322)]

_NC_CACHE = {}


def _build_nc():
    import concourse.bacc as bacc
    import concourse.mybir as mybir
    import concourse.tile as tile

    f32 = mybir.dt.float32
    f16 = mybir.dt.float16
    mult = mybir.AluOpType.mult
    add = mybir.AluOpType.add

    nc = bacc.Bacc("TRN2", target_bir_lowering=False, debug=True)

    xd = nc.dram_tensor("x", [BLOC, C, H, W], f16, kind="ExternalInput")
    # wab packs the 6 stage-A weight tiles (h, chunk) then the 4 stage-D
    # tiles (fc): [10, 128, 128] fp16, loaded in ONE dma
    wabd = nc.dram_tensor("wab", [10, 128, 128], f16, kind="ExternalInput")
    wcd = nc.dram_tensor("wc", [R, FW], f32, kind="ExternalInput")
    od = nc.dram_tensor("out", [BLOC, F, OH, OW], f16, kind="ExternalOutput")

    with tile.TileContext(nc) as tc:
        with (
            tc.tile_pool(name="wpool", bufs=1) as wpool,
            tc.tile_pool(name="xs", bufs=3) as xs_pool,
            tc.tile_pool(name="y3", bufs=3) as y3_pool,
            tc.tile_pool(name="osb", bufs=2) as osb_pool,
            tc.tile_pool(name="psA", bufs=2, space="PSUM") as psA,
            tc.tile_pool(name="psD", bufs=2, space="PSUM") as psD,
        ):
            wc_sb = wpool.tile([128, FW], f32)
            nc.sync.dma_start(wc_sb[:], wcd[:])
            wab_sb = wpool.tile([128, 10, 128], f16)
            nc.sync.dma_start(wab_sb[:], wabd.ap().rearrange("t p c -> p t c"))
            wb_off = FH * 2  # wab_sb[:, wb_off+fc, :] for stage D

            copy_i = 0  # stage-D PSUM->SBUF copy split: 3/10 DVE, 7/10 ACT

            def psum_copy(dst, src):
                nonlocal copy_i
                if copy_i % 10 in (0, 3, 6):
                    nc.vector.tensor_copy(dst, src)
                else:
                    nc.scalar.copy(dst, src)
                copy_i += 1

            def stage_a_c(xs_t, y3_t):
                """Stage A+B matmuls and stage C w-conv for one half-image."""
                for r0, nr in ROW_TILES:
                    pa = psA.tile([128, 2, 512], f32)
                    hrs = (5, nr - 5)  # rows per bank
                    for j in (0, 1):
                        hr = hrs[j]
                        k = 0
                        for h in range(FH):
                            for ch in range(2):
                                x0 = (r0 + 5 * j + h) * W
                                nc.tensor.matmul(
                                    pa[:, j, 0 : hr * W],
                                    wab_sb[:, h * 2 + ch, :],
                                    xs_t[:, ch, x0 : x0 + hr * W],
                                    start=(k == 0),
                                    stop=(k == 5),
                                )
                                k += 1
                    # stage C: one triple per PSUM bank ([p, rows, 94] APs —
                    # the ISA caps TensorScalarPtr/Activation inputs at 3D).
                    groups = []
                    for j in (0, 1):
                        hr = hrs[j]
                        s3 = pa[:, j, 0 : hr * W].rearrange(
                            "p (r c) -> p r c", c=W)
                        groups.append(
                            (lambda w, s=s3: s[:, :, w : w + OW],
                             r0 + 5 * j, hr)
                        )
                    for tap, rr, rn in groups:
                        dst = y3_t[:, rr * OW : (rr + rn) * OW]
                        nc.scalar.mul(dst, tap(0), wc_sb[:, 0:1])
                        nc.vector.scalar_tensor_tensor(
                            dst, tap(1), wc_sb[:, 1:2], dst,
                            op0=mult, op1=add,
                        )
                        nc.vector.scalar_tensor_tensor(
                            dst, tap(2), wc_sb[:, 2:3], dst,
                            op0=mult, op1=add,
                        )

            def stage_d(b, half, y3_t):
                """Stage D projection + PSUM->SBUF copies + output DMAs."""
                i0 = half * SH
                ot = osb_pool.tile([128, 4, SH * OW], f16)
                for fc in range(4):
                    for c0, cw in COL_CHUNKS:
                        pd = psD.tile([128, 2, 512], f32)
                        for j in range((cw + 511) // 512):
                            w_ = min(512, cw - j * 512)
                            nc.tensor.matmul(
                                pd[:, j, 0:w_],
                                wab_sb[:, wb_off + fc, :],
                                y3_t[:, c0 + j * 512 : c0 + j * 512 + w_],
                                start=True,
                                stop=True,
                            )
                        if cw > 512:
                            src = pd[:, :, :]
                        else:
                            src = pd[:, 0, 0:cw]
                        psum_copy(ot[:, fc, c0 : c0 + cw], src)
                    nc.sync.dma_start(
                        od[b, fc * 128 : (fc + 1) * 128, i0 : i0 + SH, :],
                        ot[:, fc, :],
                    )

            # software pipeline: stage-D of half h runs after stage-A/C of
            # half h+1 has been emitted, so the PE instruction stream never
            # blocks on the stage-C tail of the current half.
            prev = None  # (b, half, y3_t)
            for b in range(BLOC):
                for half in range(2):
                    xs_t = xs_pool.tile([128, 2, XROWS * W], f16)
                    nc.gpsimd.dma_start(
                        xs_t[:],
                        xd[b, :, half * SH : half * SH + XROWS, :].rearrange(
                            "(t p) r c -> p t (r c)", p=128
                        ),
                    )
                    y3_t = y3_pool.tile([128, SH * OW], f16)
                    stage_a_c(xs_t, y3_t)
                    if prev is not None:
                        stage_d(*prev)
                    prev = (b, half, y3_t)
            stage_d(*prev)

    nc.compile()
    return nc


def _get_nc():
    if "nc" not in _NC_CACHE:
        _NC_CACHE["nc"] = _build_nc()
    return _NC_CACHE["nc"]


def _prep_weights(factor0, factor1, factor2, factor3):
    # wab[0:6] = stage-A tiles: [h*2+ch, c', r] = factor3[ch*128+c', r]*factor1[h, r]
    # wab[6:10] = stage-D tiles: [fc, r, f'] = factor0[fc*128+f', r]
    wa = (factor3[None, :, :] * factor1[:, None, :]).reshape(FH, 2, 128, R)
    wb = factor0.reshape(4, 128, R).transpose(0, 2, 1)
    wab = np.concatenate(
        [wa.reshape(6, 128, R), wb], axis=0
    ).astype(np.float16)
    wab = np.ascontiguousarray(wab)
    # wc[r, w] = factor2[w, r]
    wc = np.ascontiguousarray(factor2.T, dtype=np.float32)
    return wab, wc


def _prep_x(x):
    return np.ascontiguousarray(x).astype(np.float16)


def kernel(x, factor0, factor1, factor2, factor3):
    from concourse import bass_utils

    x = np.asarray(x, dtype=np.float32)
    factor0 = np.asarray(factor0, dtype=np.float32)
    factor1 = np.asarray(factor1, dtype=np.float32)
    factor2 = np.asarray(factor2, dtype=np.float32)
    factor3 = np.asarray(factor3, dtype=np.float32)

    wab, wc = _prep_weights(factor0, factor1, factor2, factor3)
    x16 = _prep_x(x)

    nc = _get_nc()
    in_maps = [
        {"x": x16[c * BLOC : (c + 1) * BLOC], "wab": wab, "wc": wc}
        for c in range(NCORES)
    ]
    res = bass_utils.run_bass_kernel_spmd(nc, in_maps, list(range(NCORES)))
    out = np.concatenate(
        [res.results[c]["out"] for c in range(NCORES)], axis=0
    )
    return out.astype(np.float32)


# revision 7
# speedup vs baseline: 1.2634x; 1.2634x over previous
"""CP-decomposed conv (pointwise -> depthwise-h -> depthwise-w -> pointwise)
as a Bass/Tile kernel on 8 TRN2 NeuronCores.

Strategy:
  - Data-parallel over batch: 32 images -> 4 per core, no collectives.
  - fp16 wire format: x and out cross HBM as fp16; fp32 accumulation in PSUM.
  - Stage A+B: pointwise C->R with the depthwise h-conv folded in:
      y2[r,i,w] = sum_{h,c} (factor3[c,r]*factor1[h,r]) * x[c,i+h,w]
    6 accumulating fp16 matmuls per 1-bank PSUM tile (3 h-shifts x 2
    C-chunks); psA bufs=4 gives the PE three tiles of slack over stage C.
  - Stage C: depthwise w-conv as TWO DVE scalar_tensor_tensor ops by
    normalizing the taps by factor2[0,:] (folded back into the stage-D
    weights):  y3' = pa[0] + (f2[1]/f2[0])*pa[1] + (f2[2]/f2[0])*pa[2].
    No ACT involvement -> no cross-engine serial chain.
  - Stage D: projection R->F with weights factor0[f,r]*factor2[0,r]; psD
    spans 2 banks so each PSUM->SBUF copy moves 1024 elements. Copies
    split ~16/84 between DVE and ACT to balance the engines.
  - DMA: weights preshuffled on host into a contiguous per-partition
    layout (128 descriptors instead of 1280 tiny ones). Whole half-image
    input loads (2.4 MB, SWDGE on GpSimd); the very first load is split
    into row chunks so the PE starts after ~1 chunk. One output DMA per
    (half-image, f-chunk) (1.13 MB, HWDGE on SP).
  - Stage-D matmuls of half h are emitted after stage-A of half h+1 so
    the PE stream never blocks on the stage-C tail (keeps HAM warm).
"""

import sys
import numpy as np

for _p in ("/opt/trn_rl_repo",):
    if _p not in sys.path:
        sys.path.insert(0, _p)

B, C, H, W = 32, 256, 96, 96
F, FH, FW, R = 512, 3, 3, 128
OH, OW = H - FH + 1, W - FW + 1  # 94, 94
NCORES = 8
BLOC = B // NCORES  # 4 images per core

SH = 47            # output rows per half-image
XROWS = SH + 2     # x rows needed per half-image
ROW_TILES = [(r0, min(5, SH - r0)) for r0 in range(0, SH, 5)]
COL_CHUNKS = [(0, 1024), (1024, 1024), (2048, 1024), (3072, 1024), (4096, 322)]
# row chunks for the first input load (output-row tiles r0..r0+nr need x
# rows r0..r0+nr+2); non-overlapping x-row ranges
FIRST_XCHUNKS = [(0, 12), (12, 10), (22, 10), (32, 10), (42, 7)]

_NC_CACHE = {}


def _build_nc():
    import concourse.bacc as bacc
    import concourse.mybir as mybir
    import concourse.tile as tile

    f32 = mybir.dt.float32
    f16 = mybir.dt.float16
    mult = mybir.AluOpType.mult
    add = mybir.AluOpType.add

    nc = bacc.Bacc("TRN2", target_bir_lowering=False, debug=True)

    xd = nc.dram_tensor("x", [BLOC, C, H, W], f16, kind="ExternalInput")
    # wab: [p, t, c] per-partition-major packed weights (contiguous DMA):
    #   t in 0..5  -> stage-A tiles [c', r] = factor3[ch*128+c',r]*factor1[h,r]
    #   t in 6..9  -> stage-D tiles [r, f'] = factor0[fc*128+f',r]*factor2[0,r]
    wabd = nc.dram_tensor("wab", [128, 10, 128], f16, kind="ExternalInput")
    # wc: [r, j] = factor2[j+1, r] / factor2[0, r]
    wcd = nc.dram_tensor("wc", [R, 2], f32, kind="ExternalInput")
    od = nc.dram_tensor("out", [BLOC, F, OH, OW], f16, kind="ExternalOutput")

    with tile.TileContext(nc) as tc:
        with (
            tc.tile_pool(name="wpool", bufs=1) as wpool,
            tc.tile_pool(name="xs", bufs=3) as xs_pool,
            tc.tile_pool(name="y3", bufs=3) as y3_pool,
            tc.tile_pool(name="osb", bufs=2) as osb_pool,
            tc.tile_pool(name="psA", bufs=4, space="PSUM") as psA,
            tc.tile_pool(name="psD", bufs=2, space="PSUM") as psD,
        ):
            wc_sb = wpool.tile([128, 2], f32)
            nc.sync.dma_start(wc_sb[:], wcd[:])
            wab_sb = wpool.tile([128, 10, 128], f16)
            nc.sync.dma_start(wab_sb[:], wabd[:])
            wb_off = FH * 2  # wab_sb[:, wb_off+fc, :] for stage D

            copy_i = 0  # stage-D PSUM->SBUF copy split: ~30% DVE, 70% ACT

            def psum_copy(dst, src):
                nonlocal copy_i
                if copy_i % 10 in (0, 3, 6):
                    nc.vector.tensor_copy(dst, src)
                else:
                    nc.scalar.copy(dst, src)
                copy_i += 1

            def stage_a_c(xs_t, y3_t):
                """Stage A+B matmuls and 2-op stage C for one half-image."""
                for r0, nr in ROW_TILES:
                    pa = psA.tile([128, 512], f32)
                    k = 0
                    for h in range(FH):
                        for ch in range(2):
                            x0 = (r0 + h) * W
                            nc.tensor.matmul(
                                pa[:, 0 : nr * W],
                                wab_sb[:, h * 2 + ch, :],
                                xs_t[:, ch, x0 : x0 + nr * W],
                                start=(k == 0),
                                stop=(k == 5),
                            )
                            k += 1
                    # the ISA allows at most ONE PSUM input per elementwise
                    # op, so the w-conv needs 3 ops: tap0 is a plain ACT
                    # copy (factor2[0,:] is folded into the stage-D
                    # weights), taps 1-2 are DVE STTs accumulating into it.
                    s3 = pa[:, 0 : nr * W].rearrange("p (r c) -> p r c", c=W)
                    dst = y3_t[:, r0 * OW : (r0 + nr) * OW]
                    nc.scalar.copy(dst, s3[:, :, 0:OW])
                    nc.vector.scalar_tensor_tensor(
                        dst, s3[:, :, 1 : 1 + OW], wc_sb[:, 0:1], dst,
                        op0=mult, op1=add,
                    )
                    nc.vector.scalar_tensor_tensor(
                        dst, s3[:, :, 2 : 2 + OW], wc_sb[:, 1:2], dst,
                        op0=mult, op1=add,
                    )

            def stage_d(b, half, y3_t):
                """Stage D projection + PSUM->SBUF copies + output DMAs."""
                i0 = half * SH
                ot = osb_pool.tile([128, 4, SH * OW], f16)
                for fc in range(4):
                    for c0, cw in COL_CHUNKS:
                        pd = psD.tile([128, 2, 512], f32)
                        for j in range((cw + 511) // 512):
                            w_ = min(512, cw - j * 512)
                            nc.tensor.matmul(
                                pd[:, j, 0:w_],
                                wab_sb[:, wb_off + fc, :],
                                y3_t[:, c0 + j * 512 : c0 + j * 512 + w_],
                                start=True,
                                stop=True,
                            )
                        if cw > 512:
                            src = pd[:, :, :]
                        else:
                            src = pd[:, 0, 0:cw]
                        psum_copy(ot[:, fc, c0 : c0 + cw], src)
                    nc.sync.dma_start(
                        od[b, fc * 128 : (fc + 1) * 128, i0 : i0 + SH, :],
                        ot[:, fc, :],
                    )

            def load_xs(b, half, chunks=None):
                xs_t = xs_pool.tile([128, 2, XROWS * W], f16)
                if chunks is None:
                    chunks = [(0, XROWS)]
                for x0, xn in chunks:
                    nc.gpsimd.dma_start(
                        xs_t[:, :, x0 * W : (x0 + xn) * W],
                        xd[b, :, half * SH + x0 : half * SH + x0 + xn, :]
                        .rearrange("(t p) r c -> p t (r c)", p=128),
                    )
                return xs_t

            # software pipeline: stage-D of half h is emitted after
            # stage-A/C of half h+1.
            prev = None  # (b, half, y3_t)
            for b in range(BLOC):
                for half in range(2):
                    xs_t = load_xs(
                        b, half,
                        FIRST_XCHUNKS if (b == 0 and half == 0) else None,
                    )
                    y3_t = y3_pool.tile([128, SH * OW], f16)
                    stage_a_c(xs_t, y3_t)
                    if prev is not None:
                        stage_d(*prev)
                    prev = (b, half, y3_t)
            stage_d(*prev)

    nc.compile()
    return nc


def _get_nc():
    if "nc" not in _NC_CACHE:
        _NC_CACHE["nc"] = _build_nc()
    return _NC_CACHE["nc"]


def _prep_weights(factor0, factor1, factor2, factor3):
    s0 = factor2[0]  # [R] normalization tap (folded into stage-D weights)
    # stage-A tiles: [c', t=h*2+ch, r]
    wa = (factor3[None, :, :] * factor1[:, None, :]).reshape(FH, 2, 128, R)
    wa = wa.transpose(2, 0, 1, 3).reshape(128, 6, R)  # [c', (h,ch), r]
    # stage-D tiles: [r, t=fc, f'] = factor0[fc*128+f', r] * s0[r]
    wb = (factor0 * s0[None, :]).reshape(4, 128, R)
    wb = wb.transpose(2, 0, 1)  # [r, fc, f']
    wab = np.concatenate([wa, wb], axis=1).astype(np.float16)
    wab = np.ascontiguousarray(wab)  # [128, 10, 128]
    # wc[r, j] = factor2[j+1, r] / factor2[0, r]
    wc = np.ascontiguousarray(
        (factor2[1:] / s0[None, :]).T, dtype=np.float32
    )
    return wab, wc


def _prep_x(x):
    return np.ascontiguousarray(x).astype(np.float16)


def kernel(x, factor0, factor1, factor2, factor3):
    from concourse import bass_utils

    x = np.asarray(x, dtype=np.float32)
    factor0 = np.asarray(factor0, dtype=np.float32)
    factor1 = np.asarray(factor1, dtype=np.float32)
    factor2 = np.asarray(factor2, dtype=np.float32)
    factor3 = np.asarray(factor3, dtype=np.float32)

    wab, wc = _prep_weights(factor0, factor1, factor2, factor3)
    x16 = _prep_x(x)

    nc = _get_nc()
    in_maps = [
        {"x": x16[c * BLOC : (c + 1) * BLOC], "wab": wab, "wc": wc}
        for c in range(NCORES)
    ]
    res = bass_utils.run_bass_kernel_spmd(nc, in_maps, list(range(NCORES)))
    out = np.concatenate(
        [res.results[c]["out"] for c in range(NCORES)], axis=0
    )
    return out.astype(np.float32)


# revision 10
# speedup vs baseline: 1.3419x; 1.0622x over previous
"""CP-decomposed conv (pointwise -> depthwise-h -> depthwise-w -> pointwise)
as a Bass/Tile kernel on 8 TRN2 NeuronCores.

Strategy:
  - Data-parallel over batch: 32 images -> 4 per core, no collectives.
  - fp16 wire format: x and out cross HBM as fp16; fp32 accumulation in PSUM.
  - Stage A+B: pointwise C->R with the depthwise h-conv folded in:
      y2[r,i,w] = sum_{h,c} (factor3[c,r]*factor1[h,r]) * x[c,i+h,w]
    6 accumulating fp16 matmuls per 1-bank PSUM tile (3 h-shifts x 2
    C-chunks); psA bufs=4 gives the PE three tiles of slack over stage C.
  - Stage C: depthwise w-conv normalized by factor2[0,:] (folded into the
    stage-D weights) so tap0 is a plain ACT copy; taps 1-2 are DVE STTs
    (the ISA allows at most one PSUM input per elementwise op):
      y3' = pa[0] + (f2[1]/f2[0])*pa[1] + (f2[2]/f2[0])*pa[2]
  - Stage D: projection R->F with weights factor0[f,r]*factor2[0,r]; psD
    spans 2 banks so each PSUM->SBUF copy moves 1024 elements. Copies
    split ~30/70 between DVE and ACT to balance the engines.
  - DMA: weights preshuffled on host into a contiguous per-partition
    layout (128 descriptors instead of 1280 tiny ones). Whole half-image
    input loads (2.4 MB, SWDGE on GpSimd); the very first load is split
    into row chunks so the PE starts after ~1 chunk. One output DMA per
    (half-image, f-chunk) (1.13 MB, HWDGE on SP).
  - Stage-D chunk units of half h-1 are interleaved two-per-tile between
    the stage-A tiles of half h, so the PE queue always holds
    dependency-free work and the copies spread across the whole half.
"""

import sys
import numpy as np

for _p in ("/opt/trn_rl_repo",):
    if _p not in sys.path:
        sys.path.insert(0, _p)

B, C, H, W = 32, 256, 96, 96
F, FH, FW, R = 512, 3, 3, 128
OH, OW = H - FH + 1, W - FW + 1  # 94, 94
NCORES = 8
BLOC = B // NCORES  # 4 images per core

SH = 47            # output rows per half-image
XROWS = SH + 2     # x rows needed per half-image
ROW_TILES = [(r0, min(5, SH - r0)) for r0 in range(0, SH, 5)]
COL_CHUNKS = [(0, 1024), (1024, 1024), (2048, 1024), (3072, 1024), (4096, 322)]
# row chunks for the first input load (output-row tiles r0..r0+nr need x
# rows r0..r0+nr+2); non-overlapping x-row ranges
FIRST_XCHUNKS = [(0, 12), (12, 10), (22, 10), (32, 10), (42, 7)]

_NC_CACHE = {}


def _build_nc():
    import concourse.bacc as bacc
    import concourse.mybir as mybir
    import concourse.tile as tile

    f32 = mybir.dt.float32
    f16 = mybir.dt.float16
    mult = mybir.AluOpType.mult
    add = mybir.AluOpType.add

    nc = bacc.Bacc("TRN2", target_bir_lowering=False, debug=True)

    xd = nc.dram_tensor("x", [BLOC, C, H, W], f16, kind="ExternalInput")
    # wab: [p, t, c] per-partition-major packed weights (contiguous DMA):
    #   t in 0..5  -> stage-A tiles [c', r] = factor3[ch*128+c',r]*factor1[h,r]
    #   t in 6..9  -> stage-D tiles [r, f'] = factor0[fc*128+f',r]*factor2[0,r]
    wabd = nc.dram_tensor("wab", [128, 10, 128], f16, kind="ExternalInput")
    # wc: [r, j] = factor2[j+1, r] / factor2[0, r]
    wcd = nc.dram_tensor("wc", [R, 2], f32, kind="ExternalInput")
    od = nc.dram_tensor("out", [BLOC, F, OH, OW], f16, kind="ExternalOutput")

    with tile.TileContext(nc) as tc:
        with (
            tc.tile_pool(name="wpool", bufs=1) as wpool,
            tc.tile_pool(name="xs", bufs=3) as xs_pool,
            tc.tile_pool(name="y3", bufs=3) as y3_pool,
            tc.tile_pool(name="osb", bufs=2) as osb_pool,
            tc.tile_pool(name="psA", bufs=4, space="PSUM") as psA,
            tc.tile_pool(name="psD", bufs=2, space="PSUM") as psD,
        ):
            wc_sb = wpool.tile([128, 2], f32)
            nc.sync.dma_start(wc_sb[:], wcd[:])
            wab_sb = wpool.tile([128, 10, 128], f16)
            nc.sync.dma_start(wab_sb[:], wabd[:])
            wb_off = FH * 2  # wab_sb[:, wb_off+fc, :] for stage D

            copy_i = 0  # stage-D PSUM->SBUF copy split: ~30% DVE, 70% ACT

            def psum_copy(dst, src):
                nonlocal copy_i
                if copy_i % 10 in (0, 3, 6):
                    nc.vector.tensor_copy(dst, src)
                else:
                    nc.scalar.copy(dst, src)
                copy_i += 1

            def emit_a_tile(xs_t, y3_t, r0, nr):
                """Stage A+B matmuls and 3-op stage C for one 5-row tile."""
                pa = psA.tile([128, 512], f32)
                k = 0
                for h in range(FH):
                    for ch in range(2):
                        x0 = (r0 + h) * W
                        nc.tensor.matmul(
                            pa[:, 0 : nr * W],
                            wab_sb[:, h * 2 + ch, :],
                            xs_t[:, ch, x0 : x0 + nr * W],
                            start=(k == 0),
                            stop=(k == 5),
                        )
                        k += 1
                # w-conv: at most one PSUM input per elementwise op -> 3 ops.
                # tap0 is a plain ACT copy (factor2[0,:] is folded into the
                # stage-D weights); taps 1-2 are DVE STTs accumulating on it.
                s3 = pa[:, 0 : nr * W].rearrange("p (r c) -> p r c", c=W)
                dst = y3_t[:, r0 * OW : (r0 + nr) * OW]
                nc.scalar.copy(dst, s3[:, :, 0:OW])
                nc.vector.scalar_tensor_tensor(
                    dst, s3[:, :, 1 : 1 + OW], wc_sb[:, 0:1], dst,
                    op0=mult, op1=add,
                )
                nc.vector.scalar_tensor_tensor(
                    dst, s3[:, :, 2 : 2 + OW], wc_sb[:, 2 - 1 : 2], dst,
                    op0=mult, op1=add,
                )

            def d_unit_gen(b, half, y3_t):
                """Yield stage-D emitters: 20 (fc, col-chunk) units; each
                fc's output DMA is emitted right after its last chunk."""
                i0 = half * SH
                ot = osb_pool.tile([128, 4, SH * OW], f16)
                for fc in range(4):
                    for ci, (c0, cw) in enumerate(COL_CHUNKS):
                        def unit(fc=fc, ci=ci, c0=c0, cw=cw):
                            pd = psD.tile([128, 2, 512], f32)
                            for j in range((cw + 511) // 512):
                                w_ = min(512, cw - j * 512)
                                nc.tensor.matmul(
                                    pd[:, j, 0:w_],
                                    wab_sb[:, wb_off + fc, :],
                                    y3_t[:, c0 + j * 512 : c0 + j * 512 + w_],
                                    start=True,
                                    stop=True,
                                )
                            src = pd[:, :, :] if cw > 512 else pd[:, 0, 0:cw]
                            psum_copy(ot[:, fc, c0 : c0 + cw], src)
                            if ci == len(COL_CHUNKS) - 1:
                                nc.sync.dma_start(
                                    od[b, fc * 128 : (fc + 1) * 128,
                                       i0 : i0 + SH, :],
                                    ot[:, fc, :],
                                )
                        yield unit

            def load_xs(b, half, chunks=None):
                xs_t = xs_pool.tile([128, 2, XROWS * W], f16)
                if chunks is None:
                    chunks = [(0, XROWS)]
                for x0, xn in chunks:
                    nc.gpsimd.dma_start(
                        xs_t[:, :, x0 * W : (x0 + xn) * W],
                        xd[b, :, half * SH + x0 : half * SH + x0 + xn, :]
                        .rearrange("(t p) r c -> p t (r c)", p=128),
                    )
                return xs_t

            # software pipeline: stage-D chunk units of half h-1 are
            # interleaved two-per-tile between the stage-A tiles of half h,
            # so the PE queue always has dependency-free work and the
            # PSUM->SBUF copies are spread across the whole half.
            dgen = iter(())  # stage-D units of the previous half
            for b in range(BLOC):
                for half in range(2):
                    xs_t = load_xs(
                        b, half,
                        FIRST_XCHUNKS if (b == 0 and half == 0) else None,
                    )
                    y3_t = y3_pool.tile([128, SH * OW], f16)
                    for r0, nr in ROW_TILES:
                        emit_a_tile(xs_t, y3_t, r0, nr)
                        for _ in range(2):
                            u = next(dgen, None)
                            if u is not None:
                                u()
                    for u in dgen:  # drain any leftovers
                        u()
                    dgen = d_unit_gen(b, half, y3_t)
            for u in dgen:  # stage D of the final half
                u()

    nc.compile()
    return nc


def _get_nc():
    if "nc" not in _NC_CACHE:
        _NC_CACHE["nc"] = _build_nc()
    return _NC_CACHE["nc"]


def _prep_weights(factor0, factor1, factor2, factor3):
    s0 = factor2[0]  # [R] normalization tap (folded into stage-D weights)
    # stage-A tiles: [c', t=h*2+ch, r]
    wa = (factor3[None, :, :] * factor1[:, None, :]).reshape(FH, 2, 128, R)
    wa = wa.transpose(2, 0, 1, 3).reshape(128, 6, R)  # [c', (h,ch), r]
    # stage-D tiles: [r, t=fc, f'] = factor0[fc*128+f', r] * s0[r]
    wb = (factor0 * s0[None, :]).reshape(4, 128, R)
    wb = wb.transpose(2, 0, 1)  # [r, fc, f']
    wab = np.concatenate([wa, wb], axis=1).astype(np.float16)
    wab = np.ascontiguousarray(wab)  # [128, 10, 128]
    # wc[r, j] = factor2[j+1, r] / factor2[0, r]
    wc = np.ascontiguousarray(
        (factor2[1:] / s0[None, :]).T, dtype=np.float32
    )
    return wab, wc


def _prep_x(x):
    return np.ascontiguousarray(x).astype(np.float16)


def kernel(x, factor0, factor1, factor2, factor3):
    from concourse import bass_utils

    x = np.asarray(x, dtype=np.float32)
    factor0 = np.asarray(factor0, dtype=np.float32)
    factor1 = np.asarray(factor1, dtype=np.float32)
    factor2 = np.asarray(factor2, dtype=np.float32)
    factor3 = np.asarray(factor3, dtype=np.float32)

    wab, wc = _prep_weights(factor0, factor1, factor2, factor3)
    x16 = _prep_x(x)

    nc = _get_nc()
    in_maps = [
        {"x": x16[c * BLOC : (c + 1) * BLOC], "wab": wab, "wc": wc}
        for c in range(NCORES)
    ]
    res = bass_utils.run_bass_kernel_spmd(nc, in_maps, list(range(NCORES)))
    out = np.concatenate(
        [res.results[c]["out"] for c in range(NCORES)], axis=0
    )
    return out.astype(np.float32)
